# revision 1
# baseline (speedup 1.0000x reference)
"""ChebNet (K=3, 3 layers) GNN on 8 Trainium2 NeuronCores.

Strategy: node-shard across 8 cores. Per spmm: AllGather bf16 scaled features
into two HBM tables (shard split 3200/3050 so int16 dma_gather indices fit),
dma_gather edge messages (edges pre-sorted by dst block on host), scatter via
one-hot PE matmuls accumulating agg^T in PSUM, per-node D^-1/2 scaling on DVE.
Dense ChebConv matmuls run feature-major on PE. All graph preprocessing
(self-loops, degrees, edge bucketing/padding, int16 index tables) happens on
host inside kernel() as part of sharding.
"""
import numpy as np
import ml_dtypes
from contextlib import ExitStack

N_NODES = 50000
DIM = 128
N_LAYERS = 3
P = 8
BLK = 128

_CFG_FULL = dict(N=50000, SHARD=6250, SPLIT=3200)


def _preprocess(src, dst, cfg):
    N, SHARD, SPLIT = cfg["N"], cfg["SHARD"], cfg["SPLIT"]
    SPLITB = SHARD - SPLIT
    NBLK = (SHARD + BLK - 1) // BLK
    src_f = np.concatenate([src.astype(np.int64), np.arange(N, dtype=np.int64)])
    dst_f = np.concatenate([dst.astype(np.int64), np.arange(N, dtype=np.int64)])
    deg = np.bincount(dst_f, minlength=N).astype(np.float32)
    dinv = np.clip(deg, 1.0, None) ** -0.5

    owner = dst_f // SHARD
    per_core = []
    counts = np.zeros((P, NBLK, 2), np.int64)
    for c in range(P):
        m = owner == c
        s = src_f[m]
        dloc = dst_f[m] - c * SHARD
        blk = dloc // BLK
        rel = dloc % BLK
        soff = s % SHARD
        half = (soff >= SPLIT).astype(np.int64)
        tab = np.where(half == 0, (s // SHARD) * SPLIT + soff,
                       (s // SHARD) * SPLITB + soff - SPLIT)
        order = np.lexsort((tab, half, blk))
        blk, rel, half, tab = blk[order], rel[order], half[order], tab[order]
        cnt = np.bincount(blk * 2 + half, minlength=NBLK * 2).reshape(NBLK, 2)
        counts[c] = cnt
        per_core.append((rel, tab, cnt))

    Tbh = np.maximum(1, -(-counts.max(axis=0) // BLK))  # [NBLK,2] tiles/block/half
    T_A, T_B = Tbh[:, 0].copy(), Tbh[:, 1].copy()
    startA = np.concatenate([[0], np.cumsum(T_A)]).astype(np.int64)
    startB = np.concatenate([[0], np.cumsum(T_B)]).astype(np.int64)
    T_totA, T_totB = int(startA[-1]), int(startB[-1])
    T_tot = T_totA + T_totB

    gidx_list, drel_list = [], []
    for c in range(P):
        rel, tab, cnt = per_core[c]
        seg = np.concatenate([[0], np.cumsum(cnt.reshape(-1))])
        idx_stream = np.zeros(T_tot * BLK, np.int64)
        rel_stream = np.full(T_tot * BLK, 255, np.int64)
        for b in range(NBLK):
            for h in (0, 1):
                n = int(cnt[b, h])
                e0 = int(seg[b * 2 + h])
                t0 = int(startA[b]) if h == 0 else T_totA + int(startB[b])
                idx_stream[t0 * BLK:t0 * BLK + n] = tab[e0:e0 + n]
                rel_stream[t0 * BLK:t0 * BLK + n] = rel[e0:e0 + n]
        L = T_tot * BLK
        pos = np.arange(L)
        gw = np.zeros((128, T_tot * 8), np.int16)
        for g in range(8):
            gw[16 * g + pos % 16, pos // 16] = idx_stream
        relw = np.ascontiguousarray(
            rel_stream.reshape(T_tot, BLK).T).astype(np.float32)
        gidx_list.append(gw)
        drel_list.append(relw)

    meta = dict(N=N, SHARD=SHARD, SPLIT=SPLIT, SPLITB=SPLITB, NBLK=NBLK,
                LASTW=SHARD - (NBLK - 1) * BLK, ABLK=SPLIT // BLK,
                T_A=T_A.tolist(), T_B=T_B.tolist(),
                startA=startA.tolist(), startB=startB.tolist(),
                T_totA=T_totA, T_totB=T_totB, T_tot=T_tot)
    return meta, dinv, gidx_list, drel_list


def _build_program(meta):
    import concourse.tile as tile
    from concourse import bacc, mybir
    f32, bf16, i16 = mybir.dt.float32, mybir.dt.bfloat16, mybir.dt.int16
    Alu, Act = mybir.AluOpType, mybir.ActivationFunctionType

    SHARD, NBLK, LASTW = meta["SHARD"], meta["NBLK"], meta["LASTW"]
    SPLIT, SPLITB, ABLK = meta["SPLIT"], meta["SPLITB"], meta["ABLK"]
    T_A, T_B = meta["T_A"], meta["T_B"]
    startA, startB = meta["startA"], meta["startB"]
    T_totA, T_tot = meta["T_totA"], meta["T_tot"]
    GROUPS = [(q * 4, min(4, NBLK - q * 4)) for q in range((NBLK + 3) // 4)]
    # Gather calls: contiguous spans of <= MAX_CT tiles per stream. The SWDGE
    # descriptor ring holds dynamic_dma_scratch_size/16 descriptors; one call
    # emits nt*128 descriptors, so keep nt*128 <= half the ring for overlap.
    MAX_CT = 6

    def mk_calls(Tarr, starts):
        calls = []   # (tile_start, nt)
        tile2call = {}
        t_end = starts[-1]
        t = 0
        while t < t_end:
            nt = min(MAX_CT, t_end - t)
            for tt in range(t, t + nt):
                tile2call[tt] = (len(calls), tt - t)
            calls.append((t, nt))
            t += nt
        return calls, tile2call

    callsA, t2cA = mk_calls(T_A, startA)
    callsB, t2cB = mk_calls(T_B, startB)

    nc = bacc.Bacc("TRN2", target_bir_lowering=False, debug=False,
                   enable_asserts=True, num_devices=P,
                   dynamic_dma_scratch_size=24576)

    def inp(name, shape, dt):
        return nc.dram_tensor(name, shape, dt, kind="ExternalInput")

    wbc = inp("wbc", [128, SHARD], f32)
    dbc_d = inp("dbc", [128, SHARD], f32)
    dnode_d = inp("dnode", [128, NBLK], f32)
    gidx_d = inp("gidx", [128, T_tot * 8], i16)
    drel_d = inp("drel", [128, T_tot], f32)
    iota_d = inp("iota", [128, 128], bf16)
    ident_d = inp("ident", [128, 128], f32)
    wall_d = inp("wall", [128, N_LAYERS * 3 * 128], bf16)
    ball_d = inp("ball", [128, N_LAYERS], f32)
    linw_d = inp("linw", [128, 1], f32)
    linb_d = inp("linb", [128, 1], f32)
    predw_d = inp("predw", [128, 1], f32)
    predb_d = inp("predb", [128, 1], f32)
    out_d = nc.dram_tensor("out", [128, NBLK], f32, kind="ExternalOutput")

    ag_in_a = nc.dram_tensor("ag_in_a", [SPLIT, 128], bf16)
    ag_in_b = nc.dram_tensor("ag_in_b", [SPLITB, 128], bf16)
    tab_a = nc.dram_tensor("tab_a", [P * SPLIT, 128], bf16, addr_space="Shared")
    tab_b = nc.dram_tensor("tab_b", [P * SPLITB, 128], bf16, addr_space="Shared")

    with tile.TileContext(nc) as tc, ExitStack() as ctx:
        const = ctx.enter_context(tc.tile_pool(name="const", bufs=1))
        mpool = ctx.enter_context(tc.tile_pool(name="mpool", bufs=4))
        spool = ctx.enter_context(tc.tile_pool(name="spool", bufs=4))
        stagp = ctx.enter_context(tc.tile_pool(name="stagp", bufs=3))
        wpool = ctx.enter_context(tc.tile_pool(name="wpool", bufs=2))
        ps512 = ctx.enter_context(tc.tile_pool(name="ps512", bufs=2, space="PSUM"))
        psT = ctx.enter_context(tc.tile_pool(name="psT", bufs=3, space="PSUM"))

        def ld(name, dram, shape, dt):
            t = const.tile(shape, dt, tag=name)
            nc.sync.dma_start(t[:], dram.ap()[:, :])
            return t

        dbc = ld("dbc", dbc_d, [128, SHARD], f32)
        dnode = ld("dnode", dnode_d, [128, NBLK], f32)
        gidx = ld("gidx", gidx_d, [128, T_tot * 8], i16)
        drel = ld("drel", drel_d, [128, T_tot], f32)
        iota = ld("iota", iota_d, [128, 128], bf16)
        ident = ld("ident", ident_d, [128, 128], f32)
        wall = ld("wall", wall_d, [128, N_LAYERS * 3 * 128], bf16)
        ball = ld("ball", ball_d, [128, N_LAYERS], f32)
        linw = ld("linw", linw_d, [128, 1], f32)
        linb = ld("linb", linb_d, [128, 1], f32)
        predw = ld("predw", predw_d, [128, 1], f32)
        predb = ld("predb", predb_d, [128, 1], f32)

        X0f = const.tile([128, SHARD], f32, tag="X0f")
        Y1f = const.tile([128, SHARD], f32, tag="Y1f")
        AGG = const.tile([128, SHARD], f32, tag="AGG")
        X0h = const.tile([128, SHARD], bf16, tag="X0h")
        Y1h = const.tile([128, SHARD], bf16, tag="Y1h")
        X2h = const.tile([128, SHARD], bf16, tag="X2h")
        logs = const.tile([128, NBLK], f32, tag="logs")

        regs = {}

        def nreg(v):
            if v not in regs:
                regs[v] = nc.gpsimd.to_reg(v)
            return regs[v]

        def blk_w(b):
            return BLK if b < NBLK - 1 else LASTW

        def emit_spmm(srcf, second):
            # h_send: PE transpose + per-node dinv scale + DMA, then AllGather
            for half_id in (0, 1):
                b_lo, b_hi = (0, ABLK) if half_id == 0 else (ABLK, NBLK)
                ag_in = ag_in_a if half_id == 0 else ag_in_b
                tab = tab_a if half_id == 0 else tab_b
                for b in range(b_lo, b_hi):
                    w = blk_w(b)
                    pt = psT.tile([128, 128], f32, tag="pt")
                    nc.tensor.transpose(pt[:w, :], srcf[:, b * BLK:b * BLK + w],
                                        ident[:])
                    st = stagp.tile([128, 128], bf16, tag="st")
                    nc.vector.tensor_scalar(st[:w, :], pt[:w, :],
                                            dnode[:w, b:b + 1], None, Alu.mult)
                    r0 = b * BLK - (0 if half_id == 0 else SPLIT)
                    nc.sync.dma_start(ag_in[r0:r0 + w, :], st[:w, :])
                nc.gpsimd.collective_compute(
                    "AllGather", Alu.bypass, replica_groups=[list(range(P))],
                    ins=[ag_in.ap().opt()], outs=[tab.ap().opt()])

            # two scatter passes: A then B
            for h in (0, 1):
                tab = tab_a if h == 0 else tab_b
                Tarr = T_A if h == 0 else T_B
                starts = startA if h == 0 else startB
                tbase = 0 if h == 0 else T_totA
                calls, t2c = (callsA, t2cA) if h == 0 else (callsB, t2cB)
                Mcall = {}
                next_call = 0
                for q, (b0, nb) in enumerate(GROUPS):
                    tile_end = starts[b0 + nb]
                    while next_call < len(calls) and \
                            calls[next_call][0] < tile_end:
                        o, nt = calls[next_call]
                        M = mpool.tile([128, MAX_CT, 128], bf16, tag=f"M{h}")
                        nc.gpsimd.dma_gather(
                            out_ap=M[:, :nt, :], in_ap=tab.ap(),
                            idxs_ap=gidx[:, 8 * (tbase + o):8 * (tbase + o + nt)],
                            num_idxs=nt * 128, num_idxs_reg=nreg(nt * 128),
                            elem_size=128)
                        Mcall[next_call] = M
                        next_call += 1
                    wq = min(SHARD - b0 * BLK, nb * BLK)
                    ps = ps512.tile([128, 512], f32, tag="ps")
                    for bi, b in enumerate(range(b0, b0 + nb)):
                        lane = bi * 128
                        for t in range(Tarr[b]):
                            Tg = tbase + starts[b] + t
                            cid, loc = t2c[starts[b] + t]
                            M = Mcall[cid]
                            S = spool.tile([128, 128], bf16, tag="S")
                            nc.vector.tensor_scalar(S[:], iota[:],
                                                    drel[:, Tg:Tg + 1], None,
                                                    Alu.is_equal)
                            nc.tensor.matmul(ps[:, lane:lane + 128],
                                             M[:, loc, :], S[:],
                                             start=(t == 0),
                                             stop=(t == Tarr[b] - 1))
                    rng = slice(b0 * BLK, b0 * BLK + wq)
                    if h == 0:
                        nc.vector.tensor_copy(AGG[:, rng], ps[:, :wq])
                    else:
                        nc.vector.tensor_add(AGG[:, rng], ps[:, :wq], AGG[:, rng])
                        if not second:
                            nc.vector.tensor_mul(Y1f[:, rng], AGG[:, rng],
                                                 dbc[:, rng])
                            nc.vector.tensor_copy(Y1h[:, rng], Y1f[:, rng])
                        else:
                            nc.vector.tensor_mul(AGG[:, rng], AGG[:, rng],
                                                 dbc[:, rng])
                            nc.vector.scalar_tensor_tensor(
                                X2h[:, rng], AGG[:, rng], 2.0, X0f[:, rng],
                                Alu.mult, Alu.subtract)

        def emit_input():
            for q, (b0, nb) in enumerate(GROUPS):
                wq = min(SHARD - b0 * BLK, nb * BLK)
                rng = slice(b0 * BLK, b0 * BLK + wq)
                wt = wpool.tile([128, 512], f32, tag="wt")
                nc.sync.dma_start(wt[:, :wq], wbc.ap()[:, rng])
                nc.vector.tensor_scalar(X0f[:, rng], wt[:, :wq], linw[:, 0:1],
                                        linb[:, 0:1], Alu.mult, Alu.add)
                nc.vector.tensor_copy(X0h[:, rng], X0f[:, rng])

        def emit_dense(layer):
            terms = [X0h, Y1h, X2h]
            for q, (b0, nb) in enumerate(GROUPS):
                wq = min(SHARD - b0 * BLK, nb * BLK)
                rng = slice(b0 * BLK, b0 * BLK + wq)
                ps = ps512.tile([128, 512], f32, tag="ps")
                for k in range(3):
                    c0 = (3 * layer + k) * 128
                    nc.tensor.matmul(ps[:, :wq], wall[:, c0:c0 + 128],
                                     terms[k][:, rng], start=(k == 0),
                                     stop=(k == 2))
                nc.scalar.activation(X0f[:, rng], ps[:, :wq], Act.Relu,
                                     bias=ball[:, layer:layer + 1])
                nc.vector.tensor_copy(X0h[:, rng], X0f[:, rng])

        def emit_pred():
            for b in range(NBLK):
                w = blk_w(b)
                ps = psT.tile([128, 128], f32, tag="pp")
                nc.tensor.matmul(ps[:w, 0:1], X0f[:, b * BLK:b * BLK + w],
                                 predw[:], start=True, stop=True)
                nc.scalar.activation(logs[:w, b:b + 1], ps[:w, 0:1],
                                     Act.Identity, bias=predb[:w, 0:1])
            nc.sync.dma_start(out_d.ap()[:, :], logs[:])

        emit_input()
        for layer in range(N_LAYERS):
            emit_spmm(X0f, second=False)
            emit_spmm(Y1f, second=True)
            emit_dense(layer)
        emit_pred()

    nc.compile()
    return nc


def _run(inputs, cfg, trace=False, time_runs=0):
    import time
    from concourse.bass_utils import run_bass_kernel_spmd
    N, SHARD = cfg["N"], cfg["SHARD"]
    NBLK = (SHARD + BLK - 1) // BLK

    weights = np.asarray(inputs["weights"], np.float32)
    src = np.asarray(inputs["src"])
    dst = np.asarray(inputs["dst"])
    lin_in_w = np.asarray(inputs["lin_in_w"], np.float32)
    lin_in_b = np.asarray(inputs["lin_in_b"], np.float32)
    cheb_ws = np.asarray(inputs["cheb_ws"], np.float32)
    cheb_bs = np.asarray(inputs["cheb_bs"], np.float32)
    pred_w = np.asarray(inputs["pred_w"], np.float32)
    pred_b = np.asarray(inputs["pred_b"], np.float32)

    meta, dinv, gidx_list, drel_list = _preprocess(src, dst, cfg)
    nc = _build_program(meta)

    iota = np.broadcast_to(np.arange(128, dtype=np.float32), (128, 128))
    iota = np.ascontiguousarray(iota).astype(ml_dtypes.bfloat16)
    ident = np.eye(128, dtype=np.float32)
    wall = np.zeros((128, N_LAYERS * 3 * 128), np.float32)
    for l in range(N_LAYERS):
        for k in range(3):
            w = cheb_ws[l][k * 128:(k + 1) * 128, :]
            wall[:, (3 * l + k) * 128:(3 * l + k + 1) * 128] = \
                -w if k == 1 else w
    wall = wall.astype(ml_dtypes.bfloat16)
    ball = np.ascontiguousarray(cheb_bs.T).astype(np.float32)
    shared = dict(
        iota=iota, ident=ident, wall=wall, ball=ball,
        linw=np.ascontiguousarray(lin_in_w.reshape(1, 128).T),
        linb=lin_in_b.reshape(128, 1).astype(np.float32),
        predw=pred_w.reshape(128, 1).astype(np.float32),
        predb=np.full((128, 1), float(pred_b[0]), np.float32),
    )
    in_maps = []
    for c in range(P):
        dv = dinv[c * SHARD:(c + 1) * SHARD]
        dn = np.ones(NBLK * BLK, np.float32)
        dn[:SHARD] = dv
        dn = np.ascontiguousarray(dn.reshape(NBLK, BLK).T)
        wsh = weights[c * SHARD:(c + 1) * SHARD]
        m = dict(shared)
        m["wbc"] = np.ascontiguousarray(
            np.broadcast_to(wsh, (128, SHARD))).astype(np.float32)
        m["dbc"] = np.ascontiguousarray(
            np.broadcast_to(dv, (128, SHARD))).astype(np.float32)
        m["dnode"] = dn
        m["gidx"] = gidx_list[c]
        m["drel"] = drel_list[c]
        in_maps.append(m)

    res = run_bass_kernel_spmd(nc, in_maps, core_ids=list(range(P)),
                               trace=trace)
    extra = {"run_walls": []}
    for _ in range(time_runs):
        t0 = time.time()
        res2 = run_bass_kernel_spmd(nc, in_maps, core_ids=list(range(P)),
                                    trace=False)
        extra["run_walls"].append(time.time() - t0)
    parts = []
    for c in range(P):
        o = res.results[c]["out"]  # [128, NBLK]
        parts.append(np.ascontiguousarray(o.T).reshape(-1)[:SHARD])
    logits = np.concatenate(parts).astype(np.float32)[:, None]
    return logits, res, extra


def kernel(**inputs):
    logits, _, _ = _run(inputs, _CFG_FULL, trace=False)
    return logits

